# revision 7
# baseline (speedup 1.0000x reference)
"""NF4-quantized linear + LoRA kernel for Trainium2 (Bass/Tile), 8-core SPMD.

Contract: kernel(**inputs) takes the FULL unsharded inputs
    x      [4096, 4096] float32
    codes  [4096, 4096] int32   (NF4 code indices, 0..15)
    scales [262144]     float32 (one absmax scale per 64 contiguous elements)
    lora_A [16, 4096]   float32
    lora_B [4096, 16]   float32
and returns the full output  y = x @ dequant(codes, scales).T + (x @ A.T) @ B.T * 2.0
of shape [4096, 4096] float32.

Sharding: tensor-parallel over out_features (column parallel). Each of the 8
NeuronCores gets codes/scales/lora_B rows for its 512 output columns plus a
full replica of x and lora_A, computes y_shard [4096, 512] on device, and the
shards are concatenated on the host.

Device algorithm per core (v3):
  1. x pipeline runs entirely on DMA hardware: a gpsimd (SWDGE) dma casts
     each [128, 4096] fp32 token tile to fp16 during the HBM->SBUF transfer,
     then one HWDGE xbar dma_start_transpose per tile produces x^T in
     k-chunk-major layout [128k, 32c, 128t].  No PE/ACT/DVE cycles spent.
  2. Dequantize W on-chip with a degree-7 polynomial in u=(c-7.5)/7.5 fitted
     to the NF4 codebook: ScalarE casts int32 codes to bf16 u, then a Horner
     chain of scalar_tensor_tensor ops split between VectorE and GpSimdE,
     with block scales applied by the final fused op (broadcast-expanded).
  3. The LoRA correction W' = W + 2*(B @ A) is folded into the W transpose:
     each [i,o] PSUM chunk accumulates transpose(W chunk) + A^T chunk @ (2B^T)
     in one fp32 PSUM group, drained once (ScalarE) to the resident fp16 W^T.
  4. y sweeps are chunk-granular: a (token-tile, o-block) PSUM chunk
     [128, 128] needs only that o-block of W^T, so matmuls start as soon as
     the first dequantized o-block lands; tiles arriving after W^T is
     complete use full [128, 512] PSUM sweeps.  Drains + y DMA run on the
     ScalarE HWDGE queue.
"""
import numpy as np

import concourse.bass as bass
import concourse.bacc as bacc
import concourse.mybir as mybir
import concourse.tile as tile
from concourse.masks import make_identity

dt = mybir.dt
A_ = mybir.AluOpType

NF4 = np.array([-1.0, -0.6961928009986877, -0.5250730514526367, -0.39491748809814453,
                -0.28444138169288635, -0.18477343022823334, -0.09105003625154495, 0.0,
                0.07958029955625534, 0.16093020141124725, 0.24611230194568634,
                0.33791524171829224, 0.44070982933044434, 0.5626170039176941,
                0.7229568362236023, 1.0], dtype=np.float64)

# deg-7 fit of NF4[c] against u = (c - 7.5) / 7.5;  POLY[0] = a7 ... POLY[7] = a0
POLY = np.polyfit((np.arange(16) - 7.5) / 7.5, NF4, 7)

N_CORES = 8
T, IN, OUT, R = 4096, 4096, 4096, 16
O = OUT // N_CORES          # 512 out columns per core
BLK = 64                    # quant block size
SCALING = 2.0               # lora_alpha / r
N_OT = O // 128             # o-tiles per core (4)
N_IC = IN // 128            # contraction chunks (32)
SUB = 512                   # dequant sub-tile width
N_SUB = IN // SUB           # subtiles per o-tile (8)
N_TT = T // 128             # token tiles (32)

XT_D = 9                    # x^T tiles resident in SBUF
N_GP = 0                    # dequant subtiles per o-tile on GpSimd (walrus
                            # rejects TensorScalarPtr on Pool; keep 0)
W_SUBT_US = 3.4             # est. wall per dequant subtile (scheduling model)
X_TILE_US = 6.0             # est. x tile arrival cadence (HBM-bound)


def _build(nc):
    x_d = nc.dram_tensor("x", [T, IN], dt.float32, kind="ExternalInput").ap()
    codes_d = nc.dram_tensor("codes", [O, IN], dt.int32, kind="ExternalInput").ap()
    scales_d = nc.dram_tensor("scales", [O, IN // BLK], dt.float32, kind="ExternalInput").ap()
    lora_a_d = nc.dram_tensor("lora_a", [R, IN], dt.float32, kind="ExternalInput").ap()
    lora_b_d = nc.dram_tensor("lora_b", [O, R], dt.float32, kind="ExternalInput").ap()
    y_d = nc.dram_tensor("y", [T, O], dt.float32, kind="ExternalOutput").ap()

    with tile.TileContext(nc) as tc:
        with tc.tile_pool(name="persist", bufs=1) as pp:
            wt = pp.tile([128, N_IC * O], dt.float16, name="wt")
            ident = pp.tile([128, 128], dt.float16, name="ident")
            identf = pp.tile([128, 128], dt.float32, name="identf")
            make_identity(nc, ident)
            make_identity(nc, identf)

            # ---- LoRA prep: a16 [r, i] fp16;  bt [r, o] fp16 scaled by 2.0 ----
            a16 = pp.tile([R, IN], dt.float16, name="a16")
            bt = pp.tile([R, O], dt.float16, name="bt")
            with tc.tile_pool(name="lora", bufs=1) as lp, \
                 tc.tile_pool(name="lpsum", bufs=2, space="PSUM") as lps:
                a_f = lp.tile([R, IN], dt.float32)
                nc.sync.dma_start(out=a_f, in_=lora_a_d)
                nc.scalar.copy(a16, a_f)
                b_f = lp.tile([128, N_OT * R], dt.float32)
                for b in range(N_OT):
                    nc.sync.dma_start(out=b_f[:, b * R:(b + 1) * R],
                                      in_=lora_b_d[b * 128:(b + 1) * 128, :])
                b16 = lp.tile([128, N_OT * R], dt.float16)
                nc.scalar.copy(b16, b_f)
                for b in range(N_OT):
                    pst = lps.tile([R, 128], dt.float16)
                    nc.tensor.transpose(pst, b16[:, b * R:(b + 1) * R], ident)
                    nc.scalar.activation(bt[:, b * 128:(b + 1) * 128], pst,
                                         mybir.ActivationFunctionType.Copy,
                                         scale=SCALING)

            with tc.tile_pool(name="wsc", bufs=1) as wsp, \
                 tc.tile_pool(name="cod", bufs=8) as cp, \
                 tc.tile_pool(name="deq", bufs=1) as dq, \
                 tc.tile_pool(name="wpr", bufs=2) as wp, \
                 tc.tile_pool(name="xb", bufs=2) as xbp, \
                 tc.tile_pool(name="xt", bufs=XT_D) as xtp, \
                 tc.tile_pool(name="tpsum", bufs=2, space="PSUM") as tps, \
                 tc.tile_pool(name="cpsum", bufs=4, space="PSUM") as cps, \
                 tc.tile_pool(name="fpsum", bufs=2, space="PSUM") as fps, \
                 tc.tile_pool(name="yo", bufs=4) as yop, \
                 tc.tile_pool(name="yf", bufs=2) as yfp:

                # ---- scales: [512, 64] fp32 -> scal fp16 [128, 4*64] ----
                scal = wsp.tile([128, N_OT * (IN // BLK)], dt.float16, name="scal")
                scal_f = wsp.tile([128, N_OT * (IN // BLK)], dt.float32, name="scal_f")
                for b in range(N_OT):
                    nc.sync.dma_start(out=scal_f[:, b * 64:(b + 1) * 64],
                                      in_=scales_d[b * 128:(b + 1) * 128, :])
                nc.scalar.copy(scal, scal_f)

                # ---------- W pieces ----------
                def codes_dma(b):
                    chunks = []
                    for g in range(N_SUB):
                        cb = cp.tile([128, SUB], dt.int32, tag="codes")
                        nc.sync.dma_start(
                            out=cb, in_=codes_d[b * 128:(b + 1) * 128,
                                                g * SUB:(g + 1) * SUB])
                        chunks.append(cb)
                    return chunks

                def u_prep(b, g, cb):
                    """ACT: u = (c - 7.5)/7.5 cast + sexp broadcast for (b,g)."""
                    u = dq.tile([128, SUB], dt.bfloat16, tag=f"u{g % 4}")
                    nc.scalar.activation(u, cb, mybir.ActivationFunctionType.Copy,
                                         bias=-1.0, scale=1.0 / 7.5)
                    sexp = dq.tile([128, SUB], dt.bfloat16, tag=f"sx{g % 2}")
                    sc = scal[:, b * 64 + g * (SUB // BLK): b * 64 + (g + 1) * (SUB // BLK)]
                    s_b = bass.AP(sc.tensor, sc.offset, [sc.ap[0], sc.ap[1], [0, BLK]])
                    nc.scalar.copy(sexp.rearrange("p (k j) -> p k j", j=BLK), s_b)
                    return u, sexp

                def horner(b, g, u, sexp, wpr, eng, pfx):
                    p = dq.tile([128, SUB], dt.bfloat16, tag=f"{pfx}p{g % 2}")
                    q = dq.tile([128, SUB], dt.bfloat16, tag=f"{pfx}q{g % 2}")
                    # walrus rejects the tensor_scalar imm variant on Pool;
                    # first multiply goes on VectorE for either engine.
                    nc.vector.tensor_scalar(p, u, float(POLY[0]), None, op0=A_.mult)
                    cur, nxt = p, q
                    for k in range(1, 7):
                        eng.scalar_tensor_tensor(nxt, cur, float(POLY[k]), u,
                                                 op0=A_.add, op1=A_.mult)
                        cur, nxt = nxt, cur
                    eng.scalar_tensor_tensor(wpr[:, g * SUB:(g + 1) * SUB], cur,
                                             float(POLY[7]), sexp,
                                             op0=A_.add, op1=A_.mult)

                def fold(b, wpr):
                    """PE: transpose wpr chunks + accumulate 2*(BA)^T; drain to wt."""
                    for grp in range(N_IC // 4):
                        tp = tps.tile([128, 4 * 128], dt.float32, tag="wtp")
                        for k in range(4):
                            c = grp * 4 + k
                            sl = slice(k * 128, (k + 1) * 128)
                            nc.tensor.matmul(tp[:, sl], wpr[:, c * 128:(c + 1) * 128],
                                             identf, is_transpose=True,
                                             start=True, stop=False)
                            nc.tensor.matmul(tp[:, sl],
                                             a16[:, c * 128:(c + 1) * 128],
                                             bt[:, b * 128:(b + 1) * 128],
                                             start=False, stop=True)
                        outap = bass.AP(wt.tensor, wt.offset + grp * 4 * O + b * 128,
                                        [wt.ap[0], [O, 4], [1, 128]])
                        nc.scalar.copy(outap, tp.rearrange("p (k f) -> p k f", k=4))

                # ---------- x pipeline ----------
                xts = {}

                def feed(t):
                    xb = xbp.tile([128, IN], dt.float16, tag="xb")
                    nc.gpsimd.dma_start(out=xb, in_=x_d[t * 128:(t + 1) * 128, :])
                    xt = xtp.tile([128, N_IC, 128], dt.float16, tag="xt")
                    nc.sync.dma_start_transpose(out=xt, in_=xb)
                    xts[t] = xt

                # ---------- y sweeps ----------
                def sweep_chunk(t, b):
                    ch = cps.tile([128, 128], dt.float32, tag="ch")
                    for c in range(N_IC):
                        nc.tensor.matmul(
                            ch, xts[t][:, c, :],
                            wt[:, c * O + b * 128: c * O + (b + 1) * 128],
                            start=(c == 0), stop=(c == N_IC - 1))
                    yo = yop.tile([128, 128], dt.float32, tag="yo")
                    nc.scalar.copy(yo, ch)
                    nc.scalar.dma_start(
                        out=y_d[t * 128:(t + 1) * 128, b * 128:(b + 1) * 128],
                        in_=yo)

                def sweep_full(t):
                    yps = fps.tile([128, O], dt.float32, tag="yps")
                    for c in range(N_IC):
                        nc.tensor.matmul(yps, xts[t][:, c, :],
                                         wt[:, c * O:(c + 1) * O],
                                         start=(c == 0), stop=(c == N_IC - 1))
                    yo = yfp.tile([128, O], dt.float32, tag="yf")
                    nc.scalar.copy(yo, yps)
                    nc.scalar.dma_start(out=y_d[t * 128:(t + 1) * 128, :], in_=yo)

                # ---------- interleaved emission ----------
                swept = [set() for _ in range(N_TT)]
                fed = 0
                # Cap W-phase feeds so a gpsimd cast-DMA can never queue ahead
                # of dequant work its xb/xt slot release depends on.
                W_FEED_CAP = min(N_TT, XT_D + 2)

                code_chunks = []
                code_chunks.append(codes_dma(0))
                while fed < 4:
                    feed(fed)
                    fed += 1
                for b in range(1, N_OT):
                    code_chunks.append(codes_dma(b))

                subt = 0          # dequant subtiles emitted (time model)
                ups_next = [u_prep(0, g, code_chunks[0][g]) for g in range(N_SUB)]
                for b in range(N_OT):
                    wpr = wp.tile([128, IN], dt.float32, tag="wpr")
                    ups = ups_next
                    # gp-assigned subtiles first: their VectorE lead-in ops
                    # run before the DVE chains, so GpSimd works in parallel.
                    g_order = [g for g in range(N_SUB) if g >= N_SUB - N_GP] + \
                              [g for g in range(N_SUB) if g < N_SUB - N_GP]
                    for g in g_order:
                        eng, pfx = ((nc.gpsimd, "g") if g >= N_SUB - N_GP
                                    else (nc.vector, "v"))
                        horner(b, g, ups[g][0], ups[g][1], wpr, eng, pfx)
                        subt += 1
                        want = min(W_FEED_CAP, 2 + int(subt * W_SUBT_US / X_TILE_US))
                        while fed < want:
                            feed(fed)
                            fed += 1
                    if b + 1 < N_OT:
                        # u/sexp for the next o-tile ahead of this fold's ACT
                        # drains, so VectorE's next Horner chain never waits.
                        ups_next = [u_prep(b + 1, g, code_chunks[b + 1][g])
                                    for g in range(N_SUB)]
                    fold(b, wpr)
                    # Cap at XT_D: slot-reusing tiles (>= XT_D) may have
                    # transposes gated on earlier tiles' final sweeps, which
                    # are only emitted post-W -- sweeping them here deadlocks
                    # the PE queue.
                    arrived = min(fed, XT_D,
                                  1 + int((subt * W_SUBT_US) / X_TILE_US))
                    for t in range(arrived):
                        if b not in swept[t]:
                            sweep_chunk(t, b)
                            swept[t].add(b)

                # ---------- post-W: finish early tiles, stream the rest ----------
                LEAD = XT_D - 1
                for t in range(N_TT):
                    while fed < min(N_TT, t + LEAD):
                        feed(fed)
                        fed += 1
                    missing = [b for b in range(N_OT) if b not in swept[t]]
                    if len(missing) == N_OT:
                        sweep_full(t)
                    else:
                        for b in missing:
                            sweep_chunk(t, b)
                    swept[t] = set(range(N_OT))
                    xts.pop(t)
    return nc


_CACHE = {}


def _get_runner():
    if "r" in _CACHE:
        return _CACHE["r"]
    nc = bacc.Bacc("TRN2", target_bir_lowering=False, debug=False)
    _build(nc)
    nc.compile()

    import jax
    from jax.experimental.shard_map import shard_map
    from jax.sharding import Mesh, PartitionSpec, NamedSharding
    from concourse.bass2jax import _bass_exec_p, partition_id_tensor, install_neuronx_cc_hook

    install_neuronx_cc_hook()
    in_names, out_names, out_avals = [], [], []
    partition_name = nc.partition_id_tensor.name if nc.partition_id_tensor else None
    for alloc in nc.m.functions[0].allocations:
        if not isinstance(alloc, mybir.MemoryLocationSet):
            continue
        name = alloc.memorylocations[0].name
        if alloc.kind == "ExternalInput":
            if name != partition_name:
                in_names.append(name)
        elif alloc.kind == "ExternalOutput":
            out_names.append(name)
            out_avals.append(jax.core.ShapedArray(tuple(alloc.tensor_shape),
                                                  mybir.dt.np(alloc.dtype)))
    n_params = len(in_names)
    all_in_names = list(in_names) + list(out_names)
    if partition_name is not None:
        all_in_names.append(partition_name)

    def _body(*args):
        operands = list(args)
        if partition_name is not None:
            operands.append(partition_id_tensor())
        return tuple(_bass_exec_p.bind(
            *operands,
            out_avals=tuple(out_avals),
            in_names=tuple(all_in_names),
            out_names=tuple(out_names),
            lowering_input_output_aliases=(),
            sim_require_finite=True,
            sim_require_nnan=True,
            nc=nc,
        ))

    devices = jax.devices()[:N_CORES]
    mesh = Mesh(np.asarray(devices), ("core",))
    n_outs = len(out_avals)
    fn = jax.jit(
        shard_map(_body, mesh=mesh,
                  in_specs=(PartitionSpec("core"),) * (n_params + n_outs),
                  out_specs=(PartitionSpec("core"),) * n_outs,
                  check_rep=False),
        donate_argnums=tuple(range(n_params, n_params + n_outs)),
        keep_unused=True)
    sharding = NamedSharding(mesh, PartitionSpec("core"))
    _CACHE["r"] = (fn, in_names, out_names, out_avals, sharding)
    return _CACHE["r"]


def kernel(x, codes, scales, lora_A, lora_B):
    import jax
    fn, in_names, out_names, out_avals, sharding = _get_runner()

    x = np.ascontiguousarray(x, dtype=np.float32)
    codes = np.ascontiguousarray(codes, dtype=np.int32)
    scales2 = np.ascontiguousarray(scales, dtype=np.float32).reshape(OUT, IN // BLK)
    lora_A = np.ascontiguousarray(lora_A, dtype=np.float32)
    lora_B = np.ascontiguousarray(lora_B, dtype=np.float32)

    per_core = {
        "x": [x] * N_CORES,
        "codes": [codes[c * O:(c + 1) * O] for c in range(N_CORES)],
        "scales": [scales2[c * O:(c + 1) * O] for c in range(N_CORES)],
        "lora_a": [lora_A] * N_CORES,
        "lora_b": [lora_B[c * O:(c + 1) * O] for c in range(N_CORES)],
    }
    concat_in = [np.concatenate(per_core[n], axis=0) for n in in_names]
    dev_in = [jax.device_put(a, sharding) for a in concat_in]
    zeros = [jax.device_put(
        np.zeros((N_CORES * av.shape[0], *av.shape[1:]), av.dtype), sharding)
        for av in out_avals]
    outs = fn(*dev_in, *zeros)
    y_all = np.asarray(outs[out_names.index("y")])  # [8*4096, 512]
    y_shards = y_all.reshape(N_CORES, T, O)
    return np.concatenate([y_shards[c] for c in range(N_CORES)], axis=1)
